# revision 13
# baseline (speedup 1.0000x reference)
"""Multi-head attention (RoPE, causal) Bass kernel for 8 TRN2 NeuronCores.

Sharding: 2-way batch x 4-way heads (4 heads per core).
Per-core inputs (DRAM, float32r unless noted):
  xT   [1024, 2048]  x[b].T
  wq/wk/wv [1024, 256]  per-head-group columns
  wo   [256, 1024]   per-head-group rows
  cdup/sdup [128, 2048] float32 RoPE tables (duplicated per stacked head pair)
  p64  [128, 128]    rotate-half partition permutation
  tri  [128, 128]    causal mask for diagonal blocks (j <= i)
Output: out [2048, 1024] partial (summed over the 4 head-group cores on host).
"""
import numpy as np
from contextlib import ExitStack

import concourse.bass as bass
import concourse.tile as tile
from concourse import bacc, mybir
from concourse.bass_utils import run_bass_kernel_spmd

D_IN = 1024
D_OUT = 1024
N_HEADS_CORE = 4          # heads per core
HD = 64                   # head dim
S = 2048                  # sequence length
B = 2
THETA = 10000.0
NCORES = 8
IS = 512                  # i-slice width
NIS = S // IS             # 4 i-slices
NJC = S // 128            # 16 j-chunks

F32 = mybir.dt.float32
F32R = mybir.dt.float32r


def build_kernel():
    nc = bacc.Bacc("TRN2", target_bir_lowering=False, debug=False)

    xT = nc.dram_tensor("xT", [D_IN, S], F32R, kind="ExternalInput").ap()
    wq = nc.dram_tensor("wq", [D_IN, 256], F32R, kind="ExternalInput").ap()
    wk = nc.dram_tensor("wk", [D_IN, 256], F32R, kind="ExternalInput").ap()
    wv = nc.dram_tensor("wv", [D_IN, 256], F32R, kind="ExternalInput").ap()
    wo = nc.dram_tensor("wo", [256, D_OUT], F32R, kind="ExternalInput").ap()
    cdup = nc.dram_tensor("cdup", [128, S], F32, kind="ExternalInput").ap()
    sdup = nc.dram_tensor("sdup", [128, S], F32, kind="ExternalInput").ap()
    p64 = nc.dram_tensor("p64", [128, 128], F32R, kind="ExternalInput").ap()
    tri = nc.dram_tensor("tri", [128, 128], F32R, kind="ExternalInput").ap()
    onesc = nc.dram_tensor("onesc", [128, 65], F32R, kind="ExternalInput").ap()
    out = nc.dram_tensor("out", [S, D_OUT], F32, kind="ExternalOutput").ap()

    with tile.TileContext(nc) as tc, ExitStack() as ctx:
        singles = ctx.enter_context(tc.tile_pool(name="singles", bufs=1))
        xpool = ctx.enter_context(tc.tile_pool(name="xpool", bufs=2))
        qk = ctx.enter_context(tc.tile_pool(name="qk", bufs=1))
        rope_tmp = ctx.enter_context(tc.tile_pool(name="rope_tmp", bufs=3))
        expp = ctx.enter_context(tc.tile_pool(name="expp", bufs=3))
        ctxp = ctx.enter_context(tc.tile_pool(name="ctxp", bufs=2))
        outp = ctx.enter_context(tc.tile_pool(name="outp", bufs=3))
        # PSUM: ps_a 4x1 bank, ps_b 2x2 banks -> 8 banks total
        ps_a = ctx.enter_context(tc.tile_pool(name="ps_a", bufs=4, space="PSUM"))
        ps_b = ctx.enter_context(tc.tile_pool(name="ps_b", bufs=2, space="PSUM"))

        # ---- constants / weights ----
        w_sb = {}
        for name, ap in (("wq", wq), ("wk", wk), ("wv", wv)):
            t = singles.tile([128, 8, 256], F32R, tag=name)
            src = bass.AP(tensor=ap.tensor, offset=0,
                          ap=[[256, 128], [128 * 256, 8], [1, 256]])
            nc.sync.dma_start(out=t, in_=src)
            w_sb[name] = t
        wo_sb = singles.tile([128, 2, 1024], F32R, tag="wo")
        nc.sync.dma_start(out=wo_sb, in_=bass.AP(
            tensor=wo.tensor, offset=0,
            ap=[[1024, 128], [128 * 1024, 2], [1, 1024]]))
        c_sb = singles.tile([128, S], F32, tag="cdup")
        nc.sync.dma_start(out=c_sb, in_=cdup)
        s_sb = singles.tile([128, S], F32, tag="sdup")
        nc.sync.dma_start(out=s_sb, in_=sdup)
        p64_sb = singles.tile([128, 128], F32R, tag="p64")
        nc.sync.dma_start(out=p64_sb, in_=p64)
        tri_sb = singles.tile([128, 128], F32R, tag="tri")
        nc.sync.dma_start(out=tri_sb, in_=tri)
        ones_sb = singles.tile([128, 65], F32R, tag="ones")
        nc.sync.dma_start(out=ones_sb, in_=onesc)

        # persistent SBUF state
        qt = [singles.tile([128, S], F32R, tag=f"qt{p}", name=f"qt{p}")
              for p in range(2)]
        kt = [singles.tile([128, S], F32R, tag=f"kt{p}", name=f"kt{p}")
              for p in range(2)]
        v4 = singles.tile([128, NJC, 4, 65], F32R, tag="v4")
        v4_ones_view = bass.AP(tensor=v4.tensor, offset=64,
                               ap=[[NJC * 4 * 65, 128], [65, NJC * 4]])
        nc.gpsimd.dma_start(out=v4_ones_view, in_=bass.AP(
            tensor=onesc.tensor, offset=64, ap=[[65, 128], [0, NJC * 4]]))
        rl = singles.tile([128, 2, IS], F32R, tag="rl")

        # ================= Phase 1: QKV projections + RoPE =================
        for it in range(NIS):
            xt_t = xpool.tile([128, 8, IS], F32R, tag="xt")
            nc.sync.dma_start(out=xt_t, in_=bass.AP(
                tensor=xT.tensor, offset=it * IS,
                ap=[[S, 128], [128 * S, 8], [1, IS]]))

            for tname, wt, dests in (("q", w_sb["wq"], qt), ("k", w_sb["wk"], kt)):
                for p in range(2):
                    proj = ps_a.tile([128, IS], F32, tag="a")
                    for c in range(8):
                        nc.tensor.matmul(proj, wt[:, c, 128 * p:128 * (p + 1)],
                                         xt_t[:, c, :],
                                         start=(c == 0), stop=(c == 7))
                    # RoPE: rot = proj * cos + (P64 @ proj) * sin'
                    raw = rope_tmp.tile([128, IS], F32R, tag="raw")
                    nc.scalar.copy(raw, proj)
                    perm = ps_b.tile([128, 2, IS], F32, tag="b")
                    nc.tensor.matmul(perm[:, 0, :], p64_sb, raw,
                                     start=True, stop=True)
                    t1 = rope_tmp.tile([128, IS], F32, tag="t1")
                    nc.vector.tensor_mul(t1, proj, c_sb[:, it * IS:(it + 1) * IS])
                    t2 = rope_tmp.tile([128, IS], F32, tag="t2")
                    nc.vector.tensor_mul(t2, perm[:, 0, :],
                                         s_sb[:, it * IS:(it + 1) * IS])
                    nc.vector.tensor_add(
                        dests[p][:, it * IS:(it + 1) * IS], t1, t2)

            # V projection: [j, 256] tiles, 4 j-subtiles per i-slice
            for half in range(2):
                vps = ps_b.tile([128, 2, 256], F32, tag="b")
                for js in range(2):
                    jt = it * 4 + half * 2 + js
                    for c in range(8):
                        nc.tensor.matmul(
                            vps[:, js, :],
                            xt_t[:, c, 128 * (half * 2 + js):128 * (half * 2 + js + 1)],
                            w_sb["wv"][:, c, :],
                            start=(c == 0), stop=(c == 7))
                for js in range(2):
                    jt = it * 4 + half * 2 + js
                    nc.vector.tensor_copy(v4[:, jt, :, 0:64],
                                          vps[:, js, :].rearrange("p (h d) -> p h d", h=4))

        # ================= Phase 2: attention + out-projection =================
        for it in range(NIS):
            ctx_tiles = {}
            for pair in range(2):
                for half in range(2):
                    h = 2 * pair + half
                    hb = 64 * half
                    qs = qt[pair][hb:hb + 64, :]
                    ks = kt[pair][hb:hb + 64, :]
                    njc = 4 * it + 4
                    ctx_ps = ps_a.tile([128, IS], F32, tag="a")
                    ctx_tiles[h] = ctx_ps
                    for q0 in range(0, njc, 2):
                        nq = min(2, njc - q0)
                        quad = ps_b.tile([128, 2, IS], F32, tag="b")
                        exps = expp.tile([128, 2, IS], F32R, tag="e")
                        for qi in range(nq):
                            jc = q0 + qi
                            c0 = max(0, 128 * (jc - 4 * it))
                            nc.tensor.matmul(
                                quad[:, qi, c0:IS],
                                ks[:, 128 * jc:128 * (jc + 1)],
                                qs[:, it * IS + c0:(it + 1) * IS],
                                start=True, stop=True)
                        nc.scalar.activation(
                            exps[:, 0:nq, :], quad[:, 0:nq, :],
                            mybir.ActivationFunctionType.Exp, scale=0.125)
                        for qi in range(nq):
                            jc = q0 + qi
                            c0 = max(0, 128 * (jc - 4 * it))
                            if c0 > 0 or jc == 4 * it:
                                # diagonal block: causal mask within [c0, c0+128)
                                nc.vector.tensor_mul(
                                    exps[:, qi, c0:c0 + 128],
                                    exps[:, qi, c0:c0 + 128], tri_sb)
                            nc.tensor.matmul(
                                ctx_ps[0:65, c0:IS],
                                v4[:, jc, h, :],
                                exps[:, qi, c0:IS],
                                start=(jc == 0), stop=(jc == njc - 1))
                    # reciprocal of l (row 64) for this head
                    with nc.allow_low_precision(reason="f32r out is fp32-width"):
                        nc.vector.reciprocal(rl[64:65, half, :],
                                             ctx_ps[64:65, :])

                # normalize both heads of the pair -> stacked ctxT in SBUF
                ctxs = ctxp.tile([128, IS], F32R, tag="c")
                bc = {}
                for half in range(2):
                    bc[half] = ps_a.tile([128, IS], F32, tag="a",
                                         name=f"bc{half}")
                    nc.tensor.matmul(bc[half][0:64, :], ones_sb[64:65, 0:64],
                                     rl[64:65, half, :],
                                     start=True, stop=True)
                for half in range(2):
                    bcs = rope_tmp.tile([64, IS], F32, tag="bcs")
                    nc.scalar.copy(bcs, bc[half][0:64, :])
                    nc.vector.tensor_mul(
                        ctxs[64 * half:64 * half + 64, :],
                        ctx_tiles[2 * pair + half][0:64, :], bcs)
                ctx_tiles[f"s{pair}"] = ctxs

            # out-projection for this i-slice
            for ib in range(4):
                ot = outp.tile([128, 1024], F32, tag="o")
                for nt in range(2):
                    ops = ps_a.tile([128, IS], F32, tag="a")
                    for pair in range(2):
                        nc.tensor.matmul(
                            ops,
                            ctx_tiles[f"s{pair}"][:, 128 * ib:128 * (ib + 1)],
                            wo_sb[:, pair, nt * IS:(nt + 1) * IS],
                            start=(pair == 0), stop=(pair == 1))
                    if nt == 0:
                        nc.vector.tensor_copy(ot[:, 0:IS], ops)
                    else:
                        nc.scalar.copy(ot[:, IS:1024], ops)
                nc.sync.dma_start(
                    out=out[it * IS + 128 * ib: it * IS + 128 * (ib + 1), :],
                    in_=ot)

    nc.compile()
    return nc


def _host_tables():
    inv_freq = 1.0 / (THETA ** (np.arange(0, HD, 2, dtype=np.float64) / HD))
    pos = np.arange(S, dtype=np.float64)
    ang = pos[None, :] * inv_freq[:, None]          # [32, S]
    cos32 = np.cos(ang).astype(np.float32)
    sin32 = np.sin(ang).astype(np.float32)
    cdup = np.concatenate([cos32, cos32, cos32, cos32], axis=0)  # [128, S]
    s_signed = np.concatenate([-sin32, sin32, -sin32, sin32], axis=0)
    p64 = np.zeros((128, 128), dtype=np.float32)
    for m in range(128):
        blk = m - (m % 64)
        d = m % 64
        p64[blk + ((d + 32) % 64), m] = 1.0
    tri = (np.arange(128)[:, None] <= np.arange(128)[None, :]).astype(np.float32)
    return cdup, s_signed, p64, tri


_NC_CACHE = {}


def kernel(x, W_q, W_k, W_v, W_o):
    x = np.ascontiguousarray(x, dtype=np.float32)
    W_q = np.ascontiguousarray(W_q, dtype=np.float32)
    W_k = np.ascontiguousarray(W_k, dtype=np.float32)
    W_v = np.ascontiguousarray(W_v, dtype=np.float32)
    W_o = np.ascontiguousarray(W_o, dtype=np.float32)

    if "nc" not in _NC_CACHE:
        _NC_CACHE["nc"] = build_kernel()
    nc = _NC_CACHE["nc"]

    cdup, sdup, p64, tri = _host_tables()
    in_maps = []
    for c in range(NCORES):
        b, g = divmod(c, 4)
        cols = slice(256 * g, 256 * (g + 1))
        in_maps.append({
            "xT": np.ascontiguousarray(x[b].T),
            "wq": np.ascontiguousarray(W_q[:, cols]),
            "wk": np.ascontiguousarray(W_k[:, cols]),
            "wv": np.ascontiguousarray(W_v[:, cols]),
            "wo": np.ascontiguousarray(W_o[cols, :]),
            "cdup": cdup, "sdup": sdup, "p64": p64, "tri": tri,
            "onesc": np.ones((128, 65), dtype=np.float32),
        })
    res = run_bass_kernel_spmd(nc, in_maps, list(range(NCORES)))
    outs = [res.results[c]["out"] for c in range(NCORES)]
    full = np.empty((B, S, D_OUT), dtype=np.float32)
    for b in range(B):
        full[b] = outs[4 * b] + outs[4 * b + 1] + outs[4 * b + 2] + outs[4 * b + 3]
    return full
